# revision 1
# baseline (speedup 1.0000x reference)
"""Trainium2 Bass kernel for nn_NeuralGeneratedConv (per-pixel generated 3x3 conv).

Contract: kernel(**inputs) takes FULL inputs (as produced by setup_inputs())
and returns the FULL [4, 16, 128, 128] float32 output. Internally the work is
sharded over 8 NeuronCores: core = batch*2 + x_half; each core handles one
batch image and a 64-column slice of the output (all 128 rows).

Per-core device program (pixels live on SBUF partitions as image rows y,
tiles iterate image columns x):
  1. net_inT[2, 8192] = coords - foa  (ACT, cast to fp32r)
  2. hT[256, 8192] = relu(W1.T @ net_inT + b1)  (PE K=2 matmuls + ACT relu)
  3. per column x0: net_out[128y, 2304] = hT_x0.T @ W2 (+ b2) in PSUM (fp32r
     matmuls, fp32 accumulate), where the 2304 axis is (o, i, dy, dx)
  4. apply: out[y, o] = sum_{i,dy,dx} net_out[y, (o,i,dy,dx)] * patch[y, (i,dy,dx)]
     via a custom DVE op computing a running cumsum of products, followed by a
     strided subtract of per-o segment boundaries. patch rows are free-dim
     slices of a y-shifted, reflect-padded image copy (Rall) built once by DMA.
"""
import numpy as np

import concourse.bass as bass
import concourse.tile as tile
from concourse import bacc, mybir
from concourse.bass_utils import run_bass_kernel_spmd

B, CI, CO, H, W, KS = 4, 16, 16, 128, 128, 3
HID = 256
NCORES = 8
XH = W // 2          # 64 columns per core
NPIX = H * XH        # 8192 pixels per core
OIQ = CO * CI * KS * KS  # 2304
PAGE = CI * KS * KS      # 144 elements per output channel
# split the 2304-wide net_out into two PSUM tiles; matmul outputs must not
# cross PSUM bank boundaries (512 fp32), so use 7+9 output channels with
# bank-aligned moving-dim slices that all stay >= 256 wide (fp32r full rate)
HALF_OS = (7, 9)                      # o's per scan half
HALF_FS = (7 * PAGE, 9 * PAGE)        # 1008, 1296 (3 PSUM banks padded)
MM_SLICES = ((512, 496), (512, 512, 272))
XPAD = XH + 2            # 66 columns per core incl. halo (host pre-padded)
YPAD = H + 2             # 130 rows incl. reflect halo (host pre-padded)
NC_PLANES = CI * KS      # 48 (i, dy) planes

_DT = mybir.dt


# --------------------------------------------------------------------------
# custom DVE op: out[p, t] = s0[p] + sum_{u<=t} in0[p, u] * in1[p, u]
# --------------------------------------------------------------------------
def _mul_cumsum_ref(in0, in1, c0, c1, c2):
    P = in0.shape[0]
    a = np.asarray(in0, np.float32).reshape(P, -1)
    b = np.asarray(in1, np.float32).reshape(P, -1)
    seed = (
        np.asarray(c0, np.float32).reshape(-1, 1)
        if isinstance(c0, np.ndarray)
        else np.float32(c0)
    )
    return (seed + np.cumsum(a * b, axis=1, dtype=np.float32)).astype(np.float32)


def _register_mul_cumsum():
    from concourse import dve_ops
    from concourse.dve_spec import Spec, Src0, Src1, C0, AluOp, scan, lower
    from concourse.dve_uop import DveOpSpec

    name = "MUL_CUMSUM_ANT"
    if name in dve_ops._SUB_OPCODE_FOR_NAME:
        return next(op for op in dve_ops.OPS if op.name == name)
    spec = Spec(body=scan(AluOp.ADD, Src0 * Src1, init=C0), reference=_mul_cumsum_ref)
    row = dve_ops._CUSTOM_DVE_ROW_BASE + len(dve_ops.OPS)
    assert row < 0x20, "custom-DVE opcode rows exhausted"
    shas = {}
    for ver in ("v3", "v4"):
        s = DveOpSpec(name=name, opcode=row, uops=lower(spec, ver=ver), rd1_en=True)
        shas[ver] = s.sha(ver)
    op = dve_ops.DveOp(name, spec, subdim=False, uops_sha=shas)
    dve_ops.OPS.append(op)
    dve_ops._SUB_OPCODE_FOR_NAME[name] = row
    dve_ops.CUSTOM_DVE_SPECS[name] = spec
    return op


# --------------------------------------------------------------------------
# device program
# --------------------------------------------------------------------------
def _build(use_b2: bool, mm_dtype: str = "float16"):
    mm_dt = getattr(_DT, mm_dtype)
    op = _register_mul_cumsum()
    nc = bacc.Bacc("TRN2", target_bir_lowering=False, debug=False)

    img = nc.dram_tensor("img", [CI, YPAD, XPAD], _DT.float32, kind="ExternalInput").ap()
    foa = nc.dram_tensor("foa", [2, 1], _DT.float32, kind="ExternalInput").ap()
    coords = nc.dram_tensor("coords", [2, NPIX], _DT.float32, kind="ExternalInput").ap()
    w1 = nc.dram_tensor("w1", [2, HID], mm_dt, kind="ExternalInput").ap()
    b1 = nc.dram_tensor("b1", [HID], _DT.float32, kind="ExternalInput").ap()
    w2 = nc.dram_tensor("w2", [HID, OIQ], mm_dt, kind="ExternalInput").ap()
    b2 = nc.dram_tensor("b2", [1, OIQ], mm_dt, kind="ExternalInput").ap()
    res = nc.dram_tensor("res", [CO, H, XH], _DT.float32, kind="ExternalOutput").ap()

    with tile.TileContext(nc) as tc:
        from contextlib import ExitStack

        ctx = ExitStack()
        with ctx:
            cp = ctx.enter_context(tc.tile_pool(name="const", bufs=1))

            # ---- persistent tiles ----
            w1_sb = [cp.tile([2, 128], mm_dt, tag=f"w1_{c}", name=f"w1sb{c}") for c in range(2)]
            b1_sb = [cp.tile([128, 1], _DT.float32, tag=f"b1_{c}", name=f"b1sb{c}") for c in range(2)]
            w2_sb = [cp.tile([128, OIQ], mm_dt, tag=f"w2_{c}", name=f"w2sb{c}") for c in range(2)]
            coords_sb = cp.tile([2, NPIX], _DT.float32, tag="coords")
            foa_sb = cp.tile([2, 1], _DT.float32, tag="foa")
            nfoa_sb = cp.tile([2, 1], _DT.float32, tag="nfoa")
            rall = cp.tile([128, NC_PLANES * XPAD], _DT.float32, tag="rall")
            ht_sb = [cp.tile([128, NPIX], mm_dt, tag=f"ht_{c}", name=f"htsb{c}") for c in range(2)]
            out_acc = cp.tile([128, CO * XH], _DT.float32, tag="out_acc")
            scratch = cp.tile([128, 16 + OIQ], _DT.float32, tag="scratch")
            if use_b2:
                b2_sb = cp.tile([1, OIQ], mm_dt, tag="b2")
                ones_sb = cp.tile([1, 128], mm_dt, tag="ones")

            # ---- input DMAs (weights pre-cast to fp16 on host) ----
            nc.sync.dma_start(coords_sb[:], coords[:])
            nc.sync.dma_start(foa_sb[:], foa[:])
            for c in range(2):
                nc.sync.dma_start(w1_sb[c][:], w1[:, c * 128:(c + 1) * 128])
                nc.sync.dma_start(b1_sb[c][:], b1[c * 128:(c + 1) * 128].unsqueeze(1))
                nc.sync.dma_start(w2_sb[c][:], w2[c * 128:(c + 1) * 128, :])
            if use_b2:
                nc.sync.dma_start(b2_sb[:], b2[:])
                nc.vector.memset(ones_sb[:], 1.0)

            # ---- Rall: y-shifted image copies (host already reflect-padded) ----
            # layout [y_part, (c, x')] with c = i*3 + dy; img is [i, y+1, x+1]
            rall_v = rall[:].rearrange("p (i d x) -> p i d x", d=KS, x=XPAD)
            for d in range(KS):  # row shift dy = d - 1
                nc.sync.dma_start(
                    rall_v[:, :, d, :], img[:, d:d + H, :].rearrange("i y x -> y i x")
                )
            rall_cx = rall[:].rearrange("p (c x) -> p c x", x=XPAD)

            # ---- zero the seed columns of the scan scratch ----
            nc.vector.memset(scratch[:, 0:16], 0.0)

            # ---- negate foa ----
            nc.vector.tensor_scalar_mul(nfoa_sb[:], foa_sb[:], -1.0)

            # ---- main loop; hT blocks (phase 1) interleaved just-in-time ----
            PB = 512
            COLS_PER_PB = PB // H  # 4
            with tc.tile_pool(name="netin", bufs=3) as nip, \
                 tc.tile_pool(name="hps", bufs=2, space="PSUM") as hps, \
                 tc.tile_pool(name="patch", bufs=2) as pp, \
                 tc.tile_pool(name="mps", bufs=2, space="PSUM") as mps:
                def emit_ht_block(pb):
                    ni = nip.tile([2, PB], mm_dt, tag="ni", name=f"ni{pb}")
                    nc.scalar.add(ni[:], coords_sb[:, pb * PB:(pb + 1) * PB], nfoa_sb[:])
                    for c in range(2):
                        ps = hps.tile([128, PB], _DT.float32, tag="hps", name=f"hps{pb}_{c}")
                        nc.tensor.matmul(ps[:], w1_sb[c][:], ni[:], start=True, stop=True)
                        nc.scalar.activation(
                            ht_sb[c][:, pb * PB:(pb + 1) * PB], ps[:],
                            mybir.ActivationFunctionType.Relu,
                            bias=b1_sb[c][:], scale=1.0,
                        )

                for x0 in range(XH):
                    if x0 % COLS_PER_PB == 0:
                        emit_ht_block(x0 // COLS_PER_PB)
                    # patch rows for this column: [y, (i, dy, dx)] -> dense [128, 144]
                    pt = pp.tile([128, PAGE], _DT.float32, tag="pt")
                    nc.scalar.copy(
                        pt[:].rearrange("p (c x) -> p c x", x=KS),
                        rall_cx[:, :, x0:x0 + KS],
                    )
                    pss = [
                        mps.tile([128, max(HALF_FS)], _DT.float32, tag="mps",
                                 name=f"mps{x0}_{hf}")
                        for hf in range(2)
                    ]
                    # kc-outer: the 5 N-slices of both halves reuse one
                    # stationary hT tile, so only 2 weight swaps per column
                    for c in range(2):
                        hf_base = 0
                        for hf in range(2):
                            off = 0
                            for nw in MM_SLICES[hf]:
                                nc.tensor.matmul(
                                    pss[hf][:, off:off + nw],
                                    ht_sb[c][:, x0 * 128:(x0 + 1) * 128],
                                    w2_sb[c][:, hf_base + off: hf_base + off + nw],
                                    start=(c == 0),
                                    stop=(c == 1 and not use_b2),
                                )
                                off += nw
                            hf_base += HALF_FS[hf]
                    if use_b2:
                        hf_base = 0
                        for hf in range(2):
                            off = 0
                            for nw in MM_SLICES[hf]:
                                nc.tensor.matmul(
                                    pss[hf][:, off:off + nw],
                                    ones_sb[:],
                                    b2_sb[:, hf_base + off: hf_base + off + nw],
                                    start=False, stop=True,
                                )
                                off += nw
                            hf_base += HALF_FS[hf]
                    hf_base = 0
                    for hf in range(2):
                        n_o, half_f = HALF_OS[hf], HALF_FS[hf]
                        pt_b = pt[:].unsqueeze(1).broadcast_to([128, n_o, PAGE])
                        nc.vector._custom_dve(
                            op,
                            out=scratch[:, 16 + hf_base:16 + hf_base + half_f],
                            in0=pss[hf][:, 0:half_f],
                            in1=pt_b,
                            s0=0.0 if hf == 0 else scratch[:, 15 + hf_base:16 + hf_base],
                        )
                        hf_base += half_f
                    # per-o sums = cumsum at page ends minus previous page end
                    ends = scratch[:, 16:16 + OIQ].rearrange(
                        "p (s n) -> p s n", n=PAGE
                    )[:, :, PAGE - 1]
                    starts = scratch[:, 15:15 + OIQ].rearrange(
                        "p (s n) -> p s n", n=PAGE
                    )[:, :, 0]
                    oa = out_acc[:].rearrange("p (o x) -> p o x", x=XH)
                    nc.gpsimd.tensor_tensor(
                        out=oa[:, :, x0], in0=ends, in1=starts,
                        op=mybir.AluOpType.subtract,
                    )

            # ---- output ----
            nc.sync.dma_start(
                res.rearrange("o y x -> y o x"),
                out_acc[:].rearrange("p (o x) -> p o x", x=XH),
            )
    nc.compile()
    return nc


_cache = {}
MM_DTYPE = "float16"


def _get_nc(use_b2: bool):
    key = (use_b2, MM_DTYPE)
    if key not in _cache:
        _cache[key] = _build(use_b2, MM_DTYPE)
    return _cache[key]


def _make_in_maps(input_data, foa_xy, W1, b1, W2, b2):
    input_data = np.ascontiguousarray(input_data, np.float32)
    foa_xy = np.asarray(foa_xy, np.float32)
    W1 = np.ascontiguousarray(W1, np.float16)
    b1 = np.ascontiguousarray(b1, np.float32)
    W2 = np.ascontiguousarray(W2, np.float16)
    b2 = np.ascontiguousarray(b2, np.float16).reshape(1, OIQ)
    # reflect-pad once: [B, CI, H+2, W+2]
    padded = np.pad(input_data, ((0, 0), (0, 0), (1, 1), (1, 1)), mode="reflect")
    in_maps = []
    for core in range(NCORES):
        b, half = divmod(core, 2)
        c0 = half * XH
        img = np.ascontiguousarray(padded[b, :, :, c0:c0 + XPAD])  # [CI, YPAD, XPAD]
        xs = np.repeat(np.arange(c0, c0 + XH, dtype=np.float32), H)
        ys = np.tile(np.arange(H, dtype=np.float32), XH)
        coords = np.stack([xs, ys], axis=0)  # [2, NPIX], pixel p = x_local*128 + y
        in_maps.append(
            dict(
                img=img,
                foa=foa_xy[b].reshape(2, 1),
                coords=np.ascontiguousarray(coords),
                w1=W1,
                b1=b1,
                w2=W2,
                b2=b2,
            )
        )
    return in_maps


def _run(inputs, trace=False, trace_cores=None):
    use_b2 = bool(np.any(np.asarray(inputs["b2"]) != 0))
    nc = _get_nc(use_b2)
    in_maps = _make_in_maps(**inputs)
    r = run_bass_kernel_spmd(
        nc, in_maps, list(range(NCORES)), trace=trace, trace_cores=trace_cores
    )
    out = np.empty((B, CO, H, W), np.float32)
    for core in range(NCORES):
        b, half = divmod(core, 2)
        out[b, :, :, half * XH:(half + 1) * XH] = r.results[core]["res"]
    return out, r


def kernel(**inputs) -> np.ndarray:
    out, _ = _run(inputs)
    return out



# revision 4
# speedup vs baseline: 1.2167x; 1.2167x over previous
"""Trainium2 Bass kernel for nn_NeuralGeneratedConv (per-pixel generated 3x3 conv).

Contract: kernel(**inputs) takes FULL inputs (as produced by setup_inputs())
and returns the FULL [4, 16, 128, 128] float32 output. Internally the work is
sharded over 8 NeuronCores: core = batch*2 + x_half; each core handles one
batch image and a 64-column slice of the output (all 128 rows).

Per-core device program (pixels live on SBUF partitions as image rows y,
iterating over the 64 image columns x):
  1. hT[j, y] per column = Relu(dyB[j,y]*b[j] + biasX[j,x0]) on the ACT
     engine, exploiting separability of the first MLP layer: z = a*dx + b*dy
     with dx constant per column and dy constant per row. No PE matmul and no
     PSUM bank for the hidden layer.
  2. net_out[y, 2304] = hT.T @ W2 in PSUM via fp16 matmuls, split into a
     B tile (o0..o5, 2 banks, single-buffered, written first) and an A tile
     (o6..o15, 3 banks, double-buffered); every matmul slice stays in one
     bank.
  3. apply: out[y, o] = sum_{i,dy,dx} net_out[y,(o,..)] * patch[y,(i,dy,dx)]
     via the custom DVE mul-cumsum scan: B first (seed 0, frees the B PSUM
     tile early), then A (seed chained from B's last element); per-o sums
     recovered by one strided ends-starts subtract on GpSimd.
  patch rows are free-dim slices of a y-shifted, reflect-padded image copy
  (rall) built once by DMA; a dense per-column patch tile is produced by the
  ACT engine 4 columns at a time via an overlapped-window access pattern.
"""
import numpy as np

import concourse.bass as bass
import concourse.tile as tile
from concourse import bacc, mybir
from concourse.bass_utils import run_bass_kernel_spmd

B, CI, CO, H, W, KS = 4, 16, 16, 128, 128, 3
HID = 256
NCORES = 8
XH = W // 2          # 64 columns per core
NPIX = H * XH        # 8192 pixels per core
OIQ = CO * CI * KS * KS  # 2304
PAGE = CI * KS * KS      # 144 elements per output channel
XPAD = XH + 2            # 66 columns per core incl. halo (host pre-padded)
YPAD = H + 2             # 130 rows incl. reflect halo (host pre-padded)
NC_PLANES = CI * KS      # 48 (i, dy) planes

# ---- apply-step split (tunable) ----
N_A = 10                 # o's in the A PSUM tile (DVE scan, double-buffered)
N_B = CO - N_A           # o's in the B PSUM tile (DVE scan, single-buffered)
FS_A = N_A * PAGE        # 1440 (3 PSUM banks)
FS_B = N_B * PAGE        # 864  (2 PSUM banks)
MM_A = (512, 512, FS_A - 1024)   # matmul N-slices, each within one bank
MM_B = (512, FS_B - 512)
PTB = 4                  # patch-copy block: columns per ACT instruction

_DT = mybir.dt


# --------------------------------------------------------------------------
# custom DVE op: out[p, t] = s0[p] + sum_{u<=t} in0[p, u] * in1[p, u]
# --------------------------------------------------------------------------
def _mul_cumsum_ref(in0, in1, c0, c1, c2):
    P = in0.shape[0]
    a = np.asarray(in0, np.float32).reshape(P, -1)
    b = np.asarray(in1, np.float32).reshape(P, -1)
    seed = (
        np.asarray(c0, np.float32).reshape(-1, 1)
        if isinstance(c0, np.ndarray)
        else np.float32(c0)
    )
    return (seed + np.cumsum(a * b, axis=1, dtype=np.float32)).astype(np.float32)


def _register_mul_cumsum():
    from concourse import dve_ops
    from concourse.dve_spec import Spec, Src0, Src1, C0, AluOp, scan, lower
    from concourse.dve_uop import DveOpSpec

    name = "MUL_CUMSUM_ANT"
    if name in dve_ops._SUB_OPCODE_FOR_NAME:
        return next(op for op in dve_ops.OPS if op.name == name)
    spec = Spec(body=scan(AluOp.ADD, Src0 * Src1, init=C0), reference=_mul_cumsum_ref)
    row = dve_ops._CUSTOM_DVE_ROW_BASE + len(dve_ops.OPS)
    assert row < 0x20, "custom-DVE opcode rows exhausted"
    shas = {}
    for ver in ("v3", "v4"):
        s = DveOpSpec(name=name, opcode=row, uops=lower(spec, ver=ver), rd1_en=True)
        shas[ver] = s.sha(ver)
    op = dve_ops.DveOp(name, spec, subdim=False, uops_sha=shas)
    dve_ops.OPS.append(op)
    dve_ops._SUB_OPCODE_FOR_NAME[name] = row
    dve_ops.CUSTOM_DVE_SPECS[name] = spec
    return op


# --------------------------------------------------------------------------
# device program
# --------------------------------------------------------------------------
def _build(use_b2: bool, mm_dtype: str = "float16"):
    mm_dt = getattr(_DT, mm_dtype)
    op = _register_mul_cumsum()
    nc = bacc.Bacc("TRN2", target_bir_lowering=False, debug=False)

    img = nc.dram_tensor("img", [CI, YPAD, XPAD], _DT.float32, kind="ExternalInput").ap()
    bsc = nc.dram_tensor("bsc", [HID, 1], _DT.float32, kind="ExternalInput").ap()
    biasx = nc.dram_tensor("biasx", [HID, XH], _DT.float32, kind="ExternalInput").ap()
    dyb = nc.dram_tensor("dyb", [128, H], _DT.float32, kind="ExternalInput").ap()
    w2 = nc.dram_tensor("w2", [HID, OIQ], mm_dt, kind="ExternalInput").ap()
    b2 = nc.dram_tensor("b2", [1, OIQ], mm_dt, kind="ExternalInput").ap()
    res = nc.dram_tensor("res", [CO, H, XH], _DT.float32, kind="ExternalOutput").ap()

    with tile.TileContext(nc) as tc:
        from contextlib import ExitStack

        ctx = ExitStack()
        with ctx:
            cp = ctx.enter_context(tc.tile_pool(name="const", bufs=1))

            # ---- persistent tiles ----
            w2_sb = [cp.tile([128, OIQ], mm_dt, tag=f"w2_{c}", name=f"w2sb{c}") for c in range(2)]
            bsc_sb = [cp.tile([128, 1], _DT.float32, tag=f"bsc_{c}", name=f"bscsb{c}") for c in range(2)]
            biasx_sb = [cp.tile([128, XH], _DT.float32, tag=f"bx_{c}", name=f"bxsb{c}") for c in range(2)]
            dyb_sb = cp.tile([128, H], _DT.float32, tag="dyb")
            rall = cp.tile([128, NC_PLANES * XPAD], _DT.float32, tag="rall")
            out_acc = cp.tile([128, CO * XH], _DT.float32, tag="out_acc")
            # double-buffered scan scratch: col 0 = zero seed, then the
            # cumsum stream over all 16 o-pages [B: o0..o5 | A: o6..o15]
            scr = [
                cp.tile([128, 1 + OIQ], _DT.float32, tag=f"scr{s}", name=f"scr{s}")
                for s in range(2)
            ]
            if use_b2:
                b2_sb = cp.tile([1, OIQ], mm_dt, tag="b2")
                ones_sb = cp.tile([1, 128], mm_dt, tag="ones")

            # ---- input DMAs (weights pre-cast/permuted on host) ----
            for c in range(2):
                nc.sync.dma_start(w2_sb[c][:], w2[c * 128:(c + 1) * 128, :])
                nc.sync.dma_start(bsc_sb[c][:], bsc[c * 128:(c + 1) * 128, :])
                nc.sync.dma_start(biasx_sb[c][:], biasx[c * 128:(c + 1) * 128, :])
            nc.sync.dma_start(dyb_sb[:], dyb[:])
            if use_b2:
                nc.sync.dma_start(b2_sb[:], b2[:])
                nc.vector.memset(ones_sb[:], 1.0)

            # ---- Rall: y-shifted image copies (host already reflect-padded) ----
            # layout [y_part, (c, x')] with c = i*3 + dy; img is [i, y+1, x+1]
            rall_v = rall[:].rearrange("p (i d x) -> p i d x", d=KS, x=XPAD)
            for d in range(KS):  # row shift dy = d - 1
                nc.sync.dma_start(
                    rall_v[:, :, d, :], img[:, d:d + H, :].rearrange("i y x -> y i x")
                )
            rall_cx = rall[:].rearrange("p (c x) -> p c x", x=XPAD)

            # ---- zero the seed columns of both scan scratches ----
            for s in range(2):
                nc.vector.memset(scr[s][:, 0:1], 0.0)

            oa = out_acc[:].rearrange("p (o x) -> p o x", x=XH)

            with tc.tile_pool(name="ht", bufs=3) as htp, \
                 tc.tile_pool(name="pt", bufs=2) as ptp, \
                 tc.tile_pool(name="aps", bufs=2, space="PSUM") as apsp, \
                 tc.tile_pool(name="bps", bufs=1, space="PSUM") as bpsp:

                def emit_ht(x0):
                    """hT[j, y] = Relu(dyB*b + biasX[:, x0]) for both halves."""
                    hts = []
                    for c in range(2):
                        ht = htp.tile([128, 128], mm_dt, tag=f"ht{c}", name=f"ht{x0}_{c}")
                        nc.scalar.activation(
                            ht[:], dyb_sb[:],
                            mybir.ActivationFunctionType.Relu,
                            bias=biasx_sb[c][:, x0:x0 + 1],
                            scale=bsc_sb[c][:, 0:1],
                        )
                        hts.append(ht)
                    return hts

                hts = emit_ht(0)
                ptt = None

                for x0 in range(XH):
                    # ---- dense patch tile, PTB columns per ACT instruction ----
                    if x0 % PTB == 0:
                        ptt = ptp.tile(
                            [128, PTB * PAGE], _DT.float32, tag="pt", name=f"pt{x0}"
                        )
                        dst = ptt[:].rearrange("p (w c x) -> p w c x", w=PTB, x=KS)
                        src = rall_cx[:, :, x0:x0 + KS].unsqueeze(1).broadcast_to(
                            [128, PTB, NC_PLANES, KS]
                        )
                        # overlapped sliding windows: stride 1 on the w dim
                        pairs = [list(p) for p in src.ap]
                        pairs[1][0] = 1
                        src.ap = mybir.VecI64Pair(pairs)
                        nc.scalar.copy(dst, src)
                    slot = x0 % PTB
                    pt_col = ptt[:, slot * PAGE:(slot + 1) * PAGE]

                    # ---- hT for the next column (one ahead) ----
                    cur_hts = hts
                    if x0 + 1 < XH:
                        hts = emit_ht(x0 + 1)

                    # ---- matmuls: B part first (frees early), then A ----
                    bps = bpsp.tile([128, FS_B], _DT.float32, tag="bps", name=f"bps{x0}")
                    aps = apsp.tile([128, FS_A], _DT.float32, tag="aps", name=f"aps{x0}")
                    for c in range(2):
                        off = 0
                        for nw in MM_B:
                            nc.tensor.matmul(
                                bps[:, off:off + nw],
                                cur_hts[c][:],
                                w2_sb[c][:, off:off + nw],
                                start=(c == 0),
                                stop=(c == 1 and not use_b2),
                            )
                            off += nw
                    for c in range(2):
                        off = 0
                        for nw in MM_A:
                            nc.tensor.matmul(
                                aps[:, off:off + nw],
                                cur_hts[c][:],
                                w2_sb[c][:, FS_B + off:FS_B + off + nw],
                                start=(c == 0),
                                stop=(c == 1 and not use_b2),
                            )
                            off += nw
                    if use_b2:
                        off = 0
                        for nw in MM_B:
                            nc.tensor.matmul(
                                bps[:, off:off + nw], ones_sb[:],
                                b2_sb[:, off:off + nw], start=False, stop=True,
                            )
                            off += nw
                        off = 0
                        for nw in MM_A:
                            nc.tensor.matmul(
                                aps[:, off:off + nw], ones_sb[:],
                                b2_sb[:, FS_B + off:FS_B + off + nw],
                                start=False, stop=True,
                            )
                            off += nw

                    # ---- DVE scans: o0..o5 from B PSUM, o6..o15 from A ----
                    s = scr[x0 % 2]
                    nc.vector._custom_dve(
                        op,
                        out=s[:, 1:1 + FS_B],
                        in0=bps[:],
                        in1=pt_col.unsqueeze(1).broadcast_to([128, N_B, PAGE]),
                        s0=0.0,
                    )
                    nc.vector._custom_dve(
                        op,
                        out=s[:, 1 + FS_B:1 + OIQ],
                        in0=aps[:],
                        in1=pt_col.unsqueeze(1).broadcast_to([128, N_A, PAGE]),
                        s0=s[:, FS_B:1 + FS_B],
                    )

                    # ---- per-o sums = ends - starts (GpSimd) ----
                    ends = s[:, 1:1 + OIQ].rearrange(
                        "p (s n) -> p s n", n=PAGE
                    )[:, :, PAGE - 1]
                    starts = s[:, 0:OIQ].rearrange(
                        "p (s n) -> p s n", n=PAGE
                    )[:, :, 0]
                    nc.gpsimd.tensor_tensor(
                        out=oa[:, :, x0], in0=ends, in1=starts,
                        op=mybir.AluOpType.subtract,
                    )

            # ---- output (host un-permutes the o axis) ----
            nc.sync.dma_start(
                res.rearrange("o y x -> y o x"),
                out_acc[:].rearrange("p (o x) -> p o x", x=XH),
            )
    nc.compile()
    return nc


_cache = {}
MM_DTYPE = "float16"


def _get_nc(use_b2: bool):
    key = (use_b2, MM_DTYPE)
    if key not in _cache:
        _cache[key] = _build(use_b2, MM_DTYPE)
    return _cache[key]


def _make_in_maps(input_data, foa_xy, W1, b1, W2, b2):
    input_data = np.ascontiguousarray(input_data, np.float32)
    foa_xy = np.asarray(foa_xy, np.float32)
    W1 = np.asarray(W1, np.float32)
    b1 = np.asarray(b1, np.float32)
    W2p = np.ascontiguousarray(W2, np.float16)
    b2p = np.ascontiguousarray(b2, np.float16).reshape(1, OIQ)
    # reflect-pad once: [B, CI, H+2, W+2]
    padded = np.pad(input_data, ((0, 0), (0, 0), (1, 1), (1, 1)), mode="reflect")
    a_vec = W1[0]
    b_vec = W1[1]
    ys = np.arange(H, dtype=np.float32)
    in_maps = []
    for core in range(NCORES):
        b, half = divmod(core, 2)
        c0 = half * XH
        fx, fy = foa_xy[b, 0], foa_xy[b, 1]
        img = np.ascontiguousarray(padded[b, :, :, c0:c0 + XPAD])  # [CI, YPAD, XPAD]
        xs = np.arange(c0, c0 + XH, dtype=np.float32)
        biasx = a_vec[:, None] * (xs - fx)[None, :] + b1[:, None]  # [256, 64]
        dyb = np.broadcast_to((ys - fy)[None, :], (128, H))        # [128, 128]
        in_maps.append(
            dict(
                img=img,
                bsc=np.ascontiguousarray(b_vec.reshape(HID, 1)),
                biasx=np.ascontiguousarray(biasx, np.float32),
                dyb=np.ascontiguousarray(dyb, np.float32),
                w2=W2p,
                b2=b2p,
            )
        )
    return in_maps


def _run(inputs, trace=False, trace_cores=None):
    use_b2 = bool(np.any(np.asarray(inputs["b2"]) != 0))
    nc = _get_nc(use_b2)
    in_maps = _make_in_maps(**inputs)
    r = run_bass_kernel_spmd(
        nc, in_maps, list(range(NCORES)), trace=trace, trace_cores=trace_cores
    )
    out = np.empty((B, CO, H, W), np.float32)
    for core in range(NCORES):
        b, half = divmod(core, 2)
        out[b, :, :, half * XH:(half + 1) * XH] = r.results[core]["res"]
    return out, r


def kernel(**inputs) -> np.ndarray:
    out, _ = _run(inputs)
    return out
